# revision 17
# baseline (speedup 1.0000x reference)
"""Trainium2 Bass kernel for batched tanh-attention flat-softmax.

Per batch b:
    Q = query[b] @ W_query; K = query[b] @ W_key      # [S, 64]
    s = tanh(Q @ K.T) * 10                            # [S, S]
    s[diag] = -inf
    out[b] = softmax(s.flatten())

Sharding: data-parallel over batch across 8 NeuronCores (6 batches/core),
W_query/W_key replicated; no cross-core communication.

Numerics: tanh(x)*10 is bounded in [-10,10], so softmax needs no max
subtraction: out = exp(10*tanh(s)) / sum(...). The diagonal is clamped to
-1e4 on the tanh output, so exp underflows to exactly 0 (matching the
reference's additive -1e8 mask).

Precision strategy: all matmuls run in bf16 hi/lo split form (1 cyc/row on
PE vs 4 for fp32) with fp32 PSUM accumulation:
  - query is split once: q = qh + ql (bf16 pair, exact to ~2^-17)
  - queryT is built by hardware DMA-transpose of the bf16 halves (free)
  - projections: [Q;K] = [Wq|Wk]h.T qh + [Wq|Wk]h.T ql + [Wq|Wk]l.T qh
  - scores: [Qh;Ql].T [Kh;Kh] + Qh.T Kl  (packed into 128 partitions)
Dropped terms are O(2^-18) relative; measured end-to-end max elementwise
error vs the fp32 reference is ~2e-3 on tiny elements, L2 ~1e-5.
"""

import numpy as np

import concourse.bass as bass
import concourse.bass_isa as bass_isa
import concourse.mybir as mybir
import concourse.tile as tile
from concourse import bacc
from concourse.bass_utils import run_bass_kernel_spmd

B = 48
S = 1024
D = 128
DK = 64
N_CORES = 8
BPC = B // N_CORES
P = 128
NQ = S // P
F32 = mybir.dt.float32
BF16 = mybir.dt.bfloat16
AL = mybir.AluOpType

TANH_CLIP = 10.0
DIAG_NEG = -1.0e4


def build_bass() -> bass.Bass:
    nc = bacc.Bacc(None, target_bir_lowering=False)

    q_d = nc.dram_tensor("query", [BPC, S, D], F32, kind="ExternalInput")
    wq_d = nc.dram_tensor("W_query", [D, DK], F32, kind="ExternalInput")
    wk_d = nc.dram_tensor("W_key", [D, DK], F32, kind="ExternalInput")
    out_d = nc.dram_tensor("out", [BPC, S, S], F32, kind="ExternalOutput")

    with tile.TileContext(nc) as tc:
        with (
            tc.tile_pool(name="singles", bufs=1) as singles,
            tc.tile_pool(name="qload", bufs=2) as qload,
            tc.tile_pool(name="qtp", bufs=2) as qtp,
            tc.tile_pool(name="projsb", bufs=2) as projsb,
            tc.tile_pool(name="tbuf", bufs=2) as tbuf,
            tc.tile_pool(name="small", bufs=2) as small,
            tc.tile_pool(name="dram", bufs=2, space="DRAM") as dpool,
            tc.tile_pool(name="ps_sc", bufs=2, space="PSUM") as ps_sc,
        ):
            # --- one-time setup ---
            # diag clamp mask: min(t, dmask) forces diagonal to -1e4
            dmask = singles.tile([P, P], F32)
            nc.vector.memset(dmask, 3.0e38)
            nc.gpsimd.affine_select(
                out=dmask,
                in_=dmask,
                compare_op=AL.not_equal,
                fill=DIAG_NEG,
                base=0,
                pattern=[[-1, P]],
                channel_multiplier=1,
            )

            # W stacked [Wq | Wk] as fp32, then bf16 hi/lo
            w32 = singles.tile([D, 2 * DK], F32)
            nc.sync.dma_start(w32[:, 0:DK], wq_d[:, :])
            nc.sync.dma_start(w32[:, DK:2 * DK], wk_d[:, :])
            wh = singles.tile([D, 2 * DK], BF16)
            nc.vector.tensor_copy(wh, w32)
            wl = singles.tile([D, 2 * DK], BF16)
            nc.vector.tensor_tensor(wl, w32, wh, AL.subtract)

            for b in range(BPC):
                # --- load query[b] as [p, n, d] (s = n*128 + p) ---
                q_sb = qload.tile([P, NQ, D], F32, tag="q")
                nc.sync.dma_start(
                    q_sb, q_d[b].rearrange("(n p) d -> p n d", p=P)
                )
                # split to bf16 hi/lo
                qh_sb = qload.tile([P, NQ, D], BF16, tag="qh")
                nc.vector.tensor_copy(qh_sb, q_sb)
                ql_sb = qload.tile([P, NQ, D], BF16, tag="ql")
                nc.vector.tensor_tensor(ql_sb, q_sb, qh_sb, AL.subtract)

                # --- queryT via DRAM roundtrip + hardware DMA transpose ---
                qhT = qtp.tile([D, S], BF16, tag="qhT")
                qlT = qtp.tile([D, S], BF16, tag="qlT")
                for src, dst, tg in ((qh_sb, qhT, "h"), (ql_sb, qlT, "l")):
                    scratch = dpool.tile([S, D], BF16, tag="scr" + tg)
                    nc.sync.dma_start(
                        scratch.rearrange("(n p) d -> p n d", p=P), src
                    )
                    nc.sync.dma_start_transpose(dst, scratch)

                # --- projections: pp[0:64]=Q, pp[64:128]=K (fp32 psum) ---
                pp = ps_sc.tile([P, 2, S], F32, tag="sc")
                ppv = pp[:, 0, :]
                for h in range(2):
                    cols = slice(h * 512, (h + 1) * 512)
                    nc.tensor.matmul(
                        ppv[:, cols], wh, qhT[:, cols], start=True, stop=False
                    )
                    nc.tensor.matmul(
                        ppv[:, cols], wh, qlT[:, cols], start=False, stop=False
                    )
                    nc.tensor.matmul(
                        ppv[:, cols], wl, qhT[:, cols], start=False, stop=True
                    )

                # --- split Q/K into bf16 hi/lo and build matmul operands ---
                hb = projsb.tile([P, S], BF16, tag="hb")   # [Qh; Kh]
                nc.vector.tensor_copy(hb, ppv)
                lb = projsb.tile([P, S], BF16, tag="lb")   # [Ql; Kl]
                nc.vector.tensor_tensor(lb, ppv, hb, AL.subtract)

                qstack = projsb.tile([P, S], BF16, tag="qstack")  # [Qh; Ql]
                nc.vector.tensor_copy(qstack[0:DK], hb[0:DK])
                nc.vector.tensor_copy(qstack[DK:P], lb[0:DK])
                khh = projsb.tile([P, S], BF16, tag="khh")        # [Kh; Kh]
                nc.vector.tensor_copy(khh[0:DK], hb[DK:P])
                nc.vector.tensor_copy(khh[DK:P], hb[DK:P])
                kl = projsb.tile([DK, S], BF16, tag="kl")         # Kl
                nc.vector.tensor_copy(kl, lb[DK:P])

                # --- scores + tanh, two 128-row chunks per PSUM tile ---
                t_sb = tbuf.tile([P, NQ, S], F32, tag="t")
                for j in range(NQ // 2):
                    sc_ps = ps_sc.tile([P, 2, S], F32, tag="sc")
                    for i in range(2):
                        qc = 2 * j + i
                        lhsT = qstack[:, qc * P:(qc + 1) * P]
                        lhsT_h = qstack[0:DK, qc * P:(qc + 1) * P]
                        for h in range(2):
                            cols = slice(h * 512, (h + 1) * 512)
                            nc.tensor.matmul(
                                sc_ps[:, i, cols], lhsT, khh[:, cols],
                                start=True, stop=False,
                            )
                            nc.tensor.matmul(
                                sc_ps[:, i, cols], lhsT_h, kl[:, cols],
                                start=False, stop=True,
                            )
                    nc.scalar.activation(
                        out=t_sb[:, 2 * j:2 * j + 2],
                        in_=sc_ps,
                        func=mybir.ActivationFunctionType.Tanh,
                    )

                # --- clamp all 8 diagonal blocks in one strided DVE op ---
                # t_sb[p, qc, qc*128 + j] for j in [0,128): free offset
                # qc*(1024+128) + j  ->  AP [[1152, 8], [1, 128]]
                diag_ap = bass.AP(
                    tensor=t_sb.tensor,
                    offset=t_sb.offset,
                    ap=[t_sb.ap[0], [S + P, NQ], [1, P]],
                )
                mask_ap = bass.AP(
                    tensor=dmask.tensor,
                    offset=dmask.offset,
                    ap=[dmask.ap[0], [0, NQ], [1, P]],
                )
                nc.vector.tensor_tensor(diag_ap, diag_ap, mask_ap, AL.min)

                # --- exp(10*t) in place, with per-partition sums ---
                rs = small.tile([P, 1], F32, tag="rs")
                nc.scalar.activation(
                    out=t_sb,
                    in_=t_sb,
                    func=mybir.ActivationFunctionType.Exp,
                    scale=TANH_CLIP,
                    accum_out=rs,
                )

                # --- Z = total sum; rz = 1/Z on all partitions ---
                zall = small.tile([P, 1], F32, tag="zall")
                nc.gpsimd.partition_all_reduce(
                    zall, rs, channels=P, reduce_op=bass_isa.ReduceOp.add
                )
                rz = small.tile([P, 1], F32, tag="rz")
                nc.vector.reciprocal(rz, zall)

                # --- normalize in place and store ---
                nc.vector.tensor_scalar_mul(t_sb, t_sb, rz)
                nc.sync.dma_start(
                    out_d[b].rearrange("(n p) s -> p n s", p=P), t_sb
                )

    nc.compile()
    return nc


_CACHED_NC = None


def kernel(**inputs: np.ndarray) -> np.ndarray:
    global _CACHED_NC
    query = np.ascontiguousarray(np.asarray(inputs["query"], dtype=np.float32))
    wq = np.ascontiguousarray(np.asarray(inputs["W_query"], dtype=np.float32))
    wk = np.ascontiguousarray(np.asarray(inputs["W_key"], dtype=np.float32))
    assert query.shape == (B, S, D), query.shape

    if _CACHED_NC is None:
        _CACHED_NC = build_bass()
    nc = _CACHED_NC

    in_maps = [
        {
            "query": query[c * BPC:(c + 1) * BPC],
            "W_query": wq,
            "W_key": wk,
        }
        for c in range(N_CORES)
    ]
    res = run_bass_kernel_spmd(nc, in_maps, core_ids=list(range(N_CORES)))
    out = np.concatenate(
        [r["out"].reshape(BPC, S * S) for r in res.results], axis=0
    )
    return out
